# revision 1
# baseline (speedup 1.0000x reference)
"""Multi-head attention (B=4, T=2048, D=768, H=12) on 8 NeuronCores.

Sharding: core c handles batch b = c//2 and head-group g = c%2 (heads
6g..6g+5).  Each core computes its 6 heads' attention and a partial
output projection (contraction over its 384 local dims of w_proj).  The
host sums the two partials per batch and adds the bias terms.

Device-side formulation (everything transposed so the contraction dim
always lands on SBUF partitions):
  xT   [768, 2048]  (host pre-transposes x[b])
  qT   = Wq_loc.T @ xT   [384, 2048]   (scaled by 1/sqrt(hd), +bias)
  kT   = Wk_loc.T @ xT   [384, 2048]   (+bias)
  v    = x @ Wv_loc      [2048, 384]   (normal layout, no bias; the
         v-bias contributes a constant row handled on host)
  S^T  = kT_h.T @ qT_h   [kpos, q]  per head
  P^T  = exp(S^T)        (scores max ~8 -> no max subtraction needed)
  O'^T = [v_h | 1]^T @ P^T  [65, q]  accumulated over kpos tiles;
         row 64 = softmax denominators
  O^T  normalized via E-matmul broadcast of reciprocal denominators
  y    = O_loc @ Wp_loc  [2048, 768]  partial (host adds partner core)

Schedule: input DMAs are chunked so PE starts within a few us; the V
projection is fused per-kpos-tile into head 0's attention sweep; the
QKV projection of pair p+1 fills PE gaps while ACT paces attention of
pair p; the dt<2 half of the output projection runs during the last
head pair's attention.
"""

import numpy as np

EMBED = 768
HEADS = 12
HD = 64
SCALE = HD ** -0.5
B, T = 4, 2048
NCORES = 8
HPC = 6            # heads per core
DL = HPC * HD      # 384 local model dims per core
USE_FP32R = True

_prog_cache = {}


def _build_program(repeat=1):
    import concourse.bass as bass
    import concourse.mybir as mybir
    import concourse.tile as tile
    from concourse import bacc

    f32 = mybir.dt.float32
    f32r = mybir.dt.float32r
    ACT_EXP = mybir.ActivationFunctionType.Exp

    fm = f32r if USE_FP32R else f32   # storage dtype of matmul operands

    nc = bacc.Bacc()

    xt_d = nc.dram_tensor("xt", [EMBED, T], fm, kind="ExternalInput")
    wq_d = nc.dram_tensor("wq", [EMBED, DL], fm, kind="ExternalInput")
    wk_d = nc.dram_tensor("wk", [EMBED, DL], fm, kind="ExternalInput")
    wv_d = nc.dram_tensor("wv", [EMBED, DL], fm, kind="ExternalInput")
    bqs_d = nc.dram_tensor("bqs", [DL], f32, kind="ExternalInput")
    bk_d = nc.dram_tensor("bk", [DL], f32, kind="ExternalInput")
    wp_d = nc.dram_tensor("wp", [DL, EMBED], fm, kind="ExternalInput")
    e2_d = nc.dram_tensor("e2", [2, 128], fm, kind="ExternalInput")
    y_d = nc.dram_tensor("y", [T, EMBED], f32, kind="ExternalOutput")

    NDT = EMBED // 128   # 6 contraction tiles over embed dim
    NKT = T // 128       # 16 key-position tiles
    NQT = T // 128       # 16 query row tiles
    QH = 2               # process queries in halves of 1024
    QHW = T // QH        # 1024

    with tile.TileContext(nc) as tc:
      for _rep in range(repeat):
        with tc.tile_pool(name="persist", bufs=1) as pers, \
             tc.tile_pool(name="qk", bufs=2) as qk_pool, \
             tc.tile_pool(name="r6p", bufs=2) as r6_pool, \
             tc.tile_pool(name="ps", bufs=2, space="PSUM") as ps_pool, \
             tc.tile_pool(name="ps_s", bufs=2, space="PSUM") as pss_pool, \
             tc.tile_pool(name="ps_o", bufs=2, space="PSUM") as pso_pool, \
             tc.tile_pool(name="pT", bufs=3) as pT_pool:
            v_sb = pers.tile([128, NKT, HPC, HD + 1], fm, name="v_sb")
            oT_sb = pers.tile([128, 3, T], fm, name="oT_sb")
            e2_sb = pers.tile([2, 128], fm, name="e2_sb")
            bqs_sb = pers.tile([128, 3], f32, name="bqs_sb")
            bk_sb = pers.tile([128, 3], f32, name="bk_sb")

            nc.sync.dma_start(out=e2_sb, in_=e2_d.ap())
            nc.gpsimd.dma_start(out=bqs_sb, in_=bqs_d.ap().rearrange("(n p) -> p n", p=128))
            nc.gpsimd.dma_start(out=bk_sb, in_=bk_d.ap().rearrange("(n p) -> p n", p=128))

            # ones column of v' (softmax denominator accumulator): fill the
            # whole tile with 1.0; the value copies below overwrite cols
            # 0:64 of each head slot, leaving col 64 = 1.0
            nc.gpsimd.memset(v_sb.bitcast(f32), 1.0)

            # PE warm-up while the input DMAs stream: dependency-free
            # matmuls on e2 ramp the PE power state before real work lands
            warm_sb = pers.tile([128, 512], fm, name="warm_sb")
            nc.vector.memset(warm_sb.bitcast(f32), 0.0)
            for wi in range(16):
                psw = ps_pool.tile([128, 512], f32, name="psw", tag="ps")
                nc.tensor.matmul(psw, warm_sb[0:2, 0:128], warm_sb[0:2, :],
                                 start=True, stop=True)

            qk_tiles = {}
            r6_tiles = {}

            def proj_qk_chunk(hp, ch, xt_sb, wq_sb, wk_sb):
                qTp, kTp = qk_tiles[hp]
                if True:
                    csl = bass.ts(ch, 512)
                    psq = ps_pool.tile([128, 512], f32, name="psq", tag="ps")
                    psk = ps_pool.tile([128, 512], f32, name="psk", tag="ps")
                    for dt in range(NDT):
                        nc.tensor.matmul(
                            psq,
                            wq_sb[:, dt, bass.ts(hp, 128)],
                            xt_sb[:, dt, csl],
                            start=(dt == 0), stop=(dt == NDT - 1),
                        )
                    for dt in range(NDT):
                        nc.tensor.matmul(
                            psk,
                            wk_sb[:, dt, bass.ts(hp, 128)],
                            xt_sb[:, dt, csl],
                            start=(dt == 0), stop=(dt == NDT - 1),
                        )
                    nc.vector.tensor_scalar(
                        out=qTp[:, csl], in0=psq,
                        scalar1=bqs_sb[:, hp:hp + 1], scalar2=float(SCALE),
                        op0=mybir.AluOpType.add, op1=mybir.AluOpType.mult,
                    )
                    nc.vector.tensor_scalar_add(
                        out=kTp[:, csl], in0=psk,
                        scalar1=bk_sb[:, hp:hp + 1],
                    )

            def proj_qk(hp, xt_sb, wq_sb, wk_sb):
                # qT/kT for head pair hp ([128, T] each, 2 heads stacked)
                qTp = qk_pool.tile([128, T], fm, name="qTp", tag="qT")
                kTp = qk_pool.tile([128, T], fm, name="kTp", tag="kT")
                qk_tiles[hp] = (qTp, kTp)
                for ch in range(4):
                    proj_qk_chunk(hp, ch, xt_sb, wq_sb, wk_sb)

            def emit_v(kt, xt_sb, wv_sb):
                # v (normal layout) for all 6 heads at kpos tile kt
                psv = ps_pool.tile([128, DL], f32, name="psv", tag="ps")
                for dt in range(NDT):
                    nc.tensor.matmul(
                        psv,
                        xt_sb[:, dt, bass.ts(kt, 128)],
                        wv_sb[:, dt, :],
                        start=(dt == 0), stop=(dt == NDT - 1),
                    )
                nc.vector.tensor_copy(
                    out=v_sb[:, kt, :, 0:HD],
                    in_=psv.rearrange("p (h d) -> p h d", h=HPC),
                )

            def attend(h, fuse_v=None, qhs=None):
                # one head: S^T -> exp -> O'^T, denominators to r6.
                # fuse_v: (xt_sb, wv_sb) to emit the V projection per kt
                # during the qh==0 sweep.
                hp, off = h // 2, (h % 2) * 64
                qTp, kTp = qk_tiles[hp]
                if h % 2 == 0 and hp not in r6_tiles:
                    r6_tiles[hp] = r6_pool.tile([2, T], fm, name="r6p", tag="r6")
                r6p = r6_tiles[hp]
                NC2 = QHW // 512
                for qh in (range(QH) if qhs is None else qhs):
                    psos = [
                        pso_pool.tile([65, 512], f32, name="pso", tag="pso")
                        for _ in range(NC2)
                    ]
                    for kt in range(NKT):
                        if fuse_v is not None and qh == 0:
                            emit_v(kt, *fuse_v)
                        pss = pss_pool.tile([128, QHW], f32, name="pss", tag="pss")
                        pT = pT_pool.tile([128, QHW], fm, name="pT", tag="pT")
                        for c2 in range(NC2):
                            nc.tensor.matmul(
                                pss[:, bass.ts(c2, 512)],
                                kTp[off:off + 64, bass.ts(kt, 128)],
                                qTp[off:off + 64, bass.ds(qh * QHW + c2 * 512, 512)],
                                start=True, stop=True,
                            )
                        nc.scalar.activation(out=pT, in_=pss, func=ACT_EXP)
                        for c2 in range(NC2):
                            nc.tensor.matmul(
                                psos[c2],
                                v_sb[:, kt, h, :],
                                pT[:, bass.ts(c2, 512)],
                                start=(kt == 0), stop=(kt == NKT - 1),
                            )
                    for c2 in range(NC2):
                        qssl = bass.ds(qh * QHW + c2 * 512, 512)
                        # engine ops need partition base in {0,32,64,96}:
                        # reciprocal at partition 64, then DMA the row down
                        # to r6's row for this head (DMA has no such limit)
                        rcp_sb = pT_pool.tile([65, 512], fm, name="rcp_sb", tag="rcp", bufs=2)
                        with nc.allow_low_precision(reason="fp32r storage"):
                            nc.vector.reciprocal(
                                out=rcp_sb[64:65, :], in_=psos[c2][64:65, :],
                            )
                        nc.sync.dma_start(
                            out=r6p[h % 2:h % 2 + 1, qssl], in_=rcp_sb[64:65, :],
                        )
                        nc.vector.tensor_copy(
                            out=oT_sb[off:off + 64, hp, qssl], in_=psos[c2][0:64, :],
                        )

            def normalize(hp):
                # oT[:, hp] *= broadcast(1/denom) via the E matmul,
                # chunked so psr rides the pss psum slots (no extra banks)
                r6p = r6_tiles[hp]
                for ch in range(4):
                    csl = bass.ts(ch, 512)
                    psr = ps_pool.tile([128, 512], f32, name="psr", tag="ps")
                    nc.tensor.matmul(
                        psr, e2_sb, r6p[:, csl],
                        start=True, stop=True,
                    )
                    nc.vector.tensor_mul(
                        out=oT_sb[:, hp, csl], in0=oT_sb[:, hp, csl], in1=psr,
                    )

            with tc.tile_pool(name="xw", bufs=1) as xw:
                xt_sb = xw.tile([128, NDT, T], fm, name="xt_sb")
                wq_sb = xw.tile([128, NDT, DL], fm, name="wq_sb")
                wk_sb = xw.tile([128, NDT, DL], fm, name="wk_sb")
                wv_sb = xw.tile([128, NDT, DL], fm, name="wv_sb")

                # chunked input DMAs: xt on the HWDGE queue, weights on the
                # SWDGE queue so they don't serialize behind xt
                nc.gpsimd.dma_start(out=wq_sb, in_=wq_d.ap().rearrange("(n p) m -> p n m", p=128))
                nc.gpsimd.dma_start(out=wk_sb, in_=wk_d.ap().rearrange("(n p) m -> p n m", p=128))
                for dt in range(NDT):
                    nc.sync.dma_start(
                        out=xt_sb[:, dt, :], in_=xt_d.ap()[bass.ts(dt, 128), :],
                    )
                nc.gpsimd.dma_start(out=wv_sb, in_=wv_d.ap().rearrange("(n p) m -> p n m", p=128))

                # startup: interleave qk-pair-0 chunks with head-0/qh0
                # attention steps (S needs only k-chunk kt//4 and q-chunk 0)
                qTp0 = qk_pool.tile([128, T], fm, name="qTp0", tag="qT")
                kTp0 = qk_pool.tile([128, T], fm, name="kTp0", tag="kT")
                qk_tiles[0] = (qTp0, kTp0)
                r6_tiles[0] = r6_pool.tile([2, T], fm, name="r6p0", tag="r6")
                pso0s = [
                    pso_pool.tile([65, 512], f32, name="pso0", tag="pso")
                    for _ in range(2)
                ]
                proj_qk_chunk(0, 0, xt_sb, wq_sb, wk_sb)
                for ch in range(1, 4):
                    proj_qk_chunk(0, ch, xt_sb, wq_sb, wk_sb)
                    for kt in range(4 * (ch - 1), 4 * (ch - 1) + (8 if ch == 3 else 4)):
                        emit_v(kt, xt_sb, wv_sb)
                        pss = pss_pool.tile([128, QHW], f32, name="pss", tag="pss")
                        pT = pT_pool.tile([128, QHW], fm, name="pT", tag="pT")
                        for c2 in range(2):
                            nc.tensor.matmul(
                                pss[:, bass.ts(c2, 512)],
                                kTp0[0:64, bass.ts(kt, 128)],
                                qTp0[0:64, bass.ts(c2, 512)],
                                start=True, stop=True,
                            )
                        nc.scalar.activation(out=pT, in_=pss, func=ACT_EXP)
                        for c2 in range(2):
                            nc.tensor.matmul(
                                pso0s[c2],
                                v_sb[:, kt, 0, :],
                                pT[:, bass.ts(c2, 512)],
                                start=(kt == 0), stop=(kt == NKT - 1),
                            )
                for c2 in range(2):
                    rcp_sb = pT_pool.tile([65, 512], fm, name="rcp_sb", tag="rcp", bufs=2)
                    with nc.allow_low_precision(reason="fp32r storage"):
                        nc.vector.reciprocal(out=rcp_sb[64:65, :], in_=pso0s[c2][64:65, :])
                    nc.sync.dma_start(
                        out=r6_tiles[0][0:1, bass.ts(c2, 512)], in_=rcp_sb[64:65, :])
                    nc.vector.tensor_copy(
                        out=oT_sb[0:64, 0, bass.ts(c2, 512)], in_=pso0s[c2][0:64, :])
                attend(0, qhs=[1])
                proj_qk(1, xt_sb, wq_sb, wk_sb)
                attend(1)
                normalize(0)
                for _q in range(QH):
                    attend(2, qhs=[_q])
                    attend(3, qhs=[_q])
                proj_qk(2, xt_sb, wq_sb, wk_sb)
                normalize(1)

            # x / qkv weights released: run the dt<2 part of the output
            # projection under the last pair's attention
            with tc.tile_pool(name="y01", bufs=1) as y01p, \
                 tc.tile_pool(name="yp", bufs=2) as ypool:
                wp_sb = ypool.tile([128, 3, EMBED], fm, name="wp_sb", bufs=1)
                nc.sync.dma_start(out=wp_sb, in_=wp_d.ap().rearrange("(n p) m -> p n m", p=128))
                y01_sb = y01p.tile([128, NQT, EMBED], f32, name="y01_sb")

                def proj01(qts):
                    for qt in qts:
                        for nh in range(2):
                            psy = ps_pool.tile([128, 512], f32, name="psy", tag="ps")
                            for dt in range(2):
                                nc.tensor.matmul(
                                    psy[:, 0:384],
                                    oT_sb[:, dt, bass.ts(qt, 128)],
                                    wp_sb[:, dt, bass.ts(nh, 384)],
                                    start=(dt == 0), stop=(dt == 1),
                                )
                            nc.vector.tensor_copy(
                                out=y01_sb[:, qt, bass.ts(nh, 384)],
                                in_=psy[:, 0:384],
                            )

                for _q in range(QH):
                    attend(4, qhs=[_q])
                    attend(5, qhs=[_q])
                    proj01(range(NQT // QH * _q, NQT // QH * (_q + 1)))

                # tail: per 512-wide chunk, normalize pair 2 then finish the
                # dt=2 projection in place and ship the output chunk
                r6p2 = r6_tiles[2]
                for c in range(4):
                    csl = bass.ts(c, 512)
                    psr = ps_pool.tile([128, 512], f32, name="psr", tag="ps")
                    nc.tensor.matmul(psr, e2_sb, r6p2[:, csl], start=True, stop=True)
                    nc.vector.tensor_mul(
                        out=oT_sb[:, 2, csl], in0=oT_sb[:, 2, csl], in1=psr,
                    )
                    for qt in range(4 * c, 4 * c + 4):
                        for nh in range(2):
                            psy2 = ps_pool.tile([128, 512], f32, name="psy2", tag="ps")
                            nc.tensor.matmul(
                                psy2[:, 0:384],
                                oT_sb[:, 2, bass.ts(qt, 128)],
                                wp_sb[:, 2, bass.ts(nh, 384)],
                                start=True, stop=True,
                            )
                            nc.vector.tensor_add(
                                out=y01_sb[:, qt, bass.ts(nh, 384)],
                                in0=y01_sb[:, qt, bass.ts(nh, 384)],
                                in1=psy2[:, 0:384],
                            )
                    for c2 in range(4):
                        cc = 4 * c + c2
                        nc.sync.dma_start(
                            out=y_d.ap()[bass.ds(128 * cc, 128), :],
                            in_=y01_sb[:, cc, :],
                        )

    nc.finalize()
    return nc


def _shard_inputs(x, w_qkv, b_qkv, w_proj):
    e2 = np.zeros((2, 128), dtype=np.float32)
    e2[0, 0:HD] = 1.0
    e2[1, HD:128] = 1.0
    in_maps = []
    for c in range(NCORES):
        b, g = c // 2, c % 2
        sl = slice(DL * g, DL * g + DL)
        in_maps.append({
            "xt": np.ascontiguousarray(x[b].T),
            "wq": np.ascontiguousarray(w_qkv[:, sl]),
            "wk": np.ascontiguousarray(w_qkv[:, EMBED:][:, sl]),
            "wv": np.ascontiguousarray(w_qkv[:, 2 * EMBED:][:, sl]),
            "bqs": np.ascontiguousarray(b_qkv[sl]),
            "bk": np.ascontiguousarray(b_qkv[EMBED:][sl]),
            "wp": np.ascontiguousarray(w_proj[sl, :]),
            "e2": e2,
        })
    return in_maps


def kernel(x, w_qkv, b_qkv, w_proj, b_proj, _profile=False, _repeat=1):
    from concourse.bass_utils import run_bass_kernel_spmd

    x = np.asarray(x, dtype=np.float32)
    w_qkv = np.asarray(w_qkv, dtype=np.float32)
    b_qkv = np.asarray(b_qkv, dtype=np.float32)
    w_proj = np.asarray(w_proj, dtype=np.float32)
    b_proj = np.asarray(b_proj, dtype=np.float32)

    if _repeat not in _prog_cache:
        _prog_cache[_repeat] = _build_program(_repeat)
    nc = _prog_cache[_repeat]

    in_maps = _shard_inputs(x, w_qkv, b_qkv, w_proj)
    res = run_bass_kernel_spmd(
        nc, in_maps, list(range(NCORES)), trace=_profile,
    )

    # host-side gather: sum the two head-group partials per batch and add
    # the bias row (v-bias folded through w_proj, plus b_proj itself)
    bias_row = b_qkv[2 * EMBED:] @ w_proj + b_proj
    y = np.empty((B, T, EMBED), dtype=np.float32)
    for b in range(B):
        y[b] = res.results[2 * b]["y"] + res.results[2 * b + 1]["y"] + bias_row
    if _profile:
        return y, res
    return y



# revision 61
# speedup vs baseline: 1.1916x; 1.1916x over previous
"""Multi-head attention (B=4, T=2048, D=768, H=12) on 8 NeuronCores.

Sharding: core c handles batch b = c//2 and head-group g = c%2 (heads
6g..6g+5).  Each core computes its 6 heads' attention and a partial
output projection (contraction over its 384 local dims of w_proj).  The
host sums the two partials per batch and adds the bias terms.

Device-side formulation:
  xT   [768, 2048]  (host pre-transposes x[b])
  qT   = Wq_loc.T @ xT   [384, 2048]   (scaled by 1/sqrt(hd), +bias)
  kT   = Wk_loc.T @ xT   [384, 2048]   (+bias)
  v    = x @ Wv_loc      [2048, 384]   (bf16 weights+output; v-bias on host)
  S^T  = kT_h.T @ qT_h   [kpos, q] per (head, kt-tile)
  P^T  = exp(S^T)        (bf16; scores max ~8 -> no max subtraction)
  O    accumulated in PSUM in [q, d] layout via stationary-P matmuls:
         O[qt] += P^T[:, qt].T @ v[kt, h]     (64-wide bf16 moving operand)
       softmax denominators via ones-column matmuls into a second PSUM
       bank, so normalization is a per-partition tensor_scalar
  oT   = PE-transpose of the normalized O (bf16, via identity matmul)
  y    = O_loc @ Wp_loc partials, shipped bf16 as two tensors (dt<2 part
         and dt=2 part); the host sums partials from both cores + biases

Schedule: ACT (exp, ~199us busy) is the pacing engine; each head's 32
exp instructions define 32 "steps".  S matmuls run 1-2 steps ahead of
the exp stream (deeper at qh boundaries where the AV matmuls wait on the
previous normalize).  The PE's per-step slack is filled with V/qk
projections, transposes and the output projection, spread so no head
exceeds the 33us exp span.
"""

import numpy as np

EMBED = 768
HEADS = 12
HD = 64
SCALE = HD ** -0.5
B, T = 4, 2048
NCORES = 8
HPC = 6            # heads per core
DL = HPC * HD      # 384 local model dims per core

_prog_cache = {}


def _build_program(repeat=1):
    import concourse.bass as bass
    import concourse.mybir as mybir
    import concourse.tile as tile
    from concourse import bacc

    f32 = mybir.dt.float32
    f32r = mybir.dt.float32r
    bf16 = mybir.dt.bfloat16
    ACT_EXP = mybir.ActivationFunctionType.Exp

    nc = bacc.Bacc()

    xt_d = nc.dram_tensor("xt", [EMBED, T], f32r, kind="ExternalInput")
    wq_d = nc.dram_tensor("wq", [EMBED, DL], f32r, kind="ExternalInput")
    wk_d = nc.dram_tensor("wk", [EMBED, DL], f32r, kind="ExternalInput")
    wv_d = nc.dram_tensor("wv", [EMBED, DL], bf16, kind="ExternalInput")
    bqs_d = nc.dram_tensor("bqs", [DL], f32, kind="ExternalInput")
    bk_d = nc.dram_tensor("bk", [DL], f32, kind="ExternalInput")
    wp_d = nc.dram_tensor("wp", [DL, EMBED], bf16, kind="ExternalInput")
    aux_d = nc.dram_tensor("aux", [128, 129], f32, kind="ExternalInput")
    y_d = nc.dram_tensor("y", [T, EMBED], bf16, kind="ExternalOutput")
    y2_d = nc.dram_tensor("y2", [T, EMBED], bf16, kind="ExternalOutput")

    NDT = EMBED // 128   # 6 contraction tiles over embed dim
    NKT = T // 128       # 16 key-position tiles
    NQT = T // 128       # 16 q tiles
    QH = 2               # q halves of 1024
    QHW = T // QH        # 1024
    QT = QHW // 128      # 8 q-tiles per half
    NST = QH * NKT       # 32 steps per head

    with tile.TileContext(nc) as tc:
      for _rep in range(repeat):
        with tc.tile_pool(name="persist", bufs=1) as pers, \
             tc.tile_pool(name="qk", bufs=2) as qk_pool, \
             tc.tile_pool(name="pT", bufs=10) as pT_pool, \
             tc.tile_pool(name="rcp", bufs=2) as rcp_pool, \
             tc.tile_pool(name="ysh", bufs=6) as ysh_pool, \
             tc.tile_pool(name="pss", bufs=2, space="PSUM") as pss_pool, \
             tc.tile_pool(name="po", bufs=1, space="PSUM") as po_pool, \
             tc.tile_pool(name="pd", bufs=1, space="PSUM") as pd_pool, \
             tc.tile_pool(name="ps", bufs=2, space="PSUM") as ps_pool:
            v_sb = pers.tile([128, NKT, HPC, HD], bf16, name="v_sb")
            oT_sb = pers.tile([128, 3, T], bf16, name="oT_sb")
            pack_sb = pers.tile([128, NQT, 128], bf16, name="pack_sb")
            eye_sb = pers.tile([128, 128], bf16, name="eye_sb")
            ones_sb = pers.tile([128, 1], bf16, name="ones_sb")
            auxf_sb = pers.tile([128, 129], f32, name="auxf_sb")
            bqs_sb = pers.tile([128, 3], f32, name="bqs_sb")
            bk_sb = pers.tile([128, 3], f32, name="bk_sb")
            warm_sb = pers.tile([128, 512], f32r, name="warm_sb")

            po_ps = po_pool.tile([128, QT, HD], f32, name="po_ps")
            pd_ps = pd_pool.tile([128, QH, QT], f32, name="pd_ps")

            # PE warm-up while the input DMAs stream: sized to keep the
            # PE busy (and the p-state ramped) until the first xt chunk
            # and weights have landed
            nc.vector.memset(warm_sb.bitcast(f32), 0.0)
            for wi in range(32):
                psw = ps_pool.tile([128, 512], f32, name="psw", tag="ps")
                nc.tensor.matmul(psw, warm_sb[0:2, 0:128], warm_sb[0:2, :],
                                 start=True, stop=True)

            qk_tiles = {}

            def new_qk(hp):
                qTp = qk_pool.tile([128, T], f32r, name="qTp", tag="qT")
                kTp = qk_pool.tile([128, T], f32r, name="kTp", tag="kT")
                qk_tiles[hp] = (qTp, kTp)

            def qk_emitters(hp, xt_sb, wq_sb, wk_sb):
                """dict {(isq, ch, half): thunk}: 3 contraction matmuls into
                a fresh one-step PSUM tile; half 0 writes qT/kT (with bias),
                half 1 accumulates on top via scalar_tensor_tensor, so no
                PSUM tile outlives its emission step."""

                def mk(isq, ch, half):
                    def f():
                        qTp, kTp = qk_tiles[hp]
                        ps = ps_pool.tile([128, 512], f32, name="psqk", tag="ps")
                        w_sb = wq_sb if isq else wk_sb
                        for dt in range(3 * half, 3 * half + 3):
                            nc.tensor.matmul(
                                ps,
                                w_sb[:, dt, bass.ts(hp, 128)],
                                xt_sb[:, ch, dt, :],
                                start=(dt == 3 * half),
                                stop=(dt == 3 * half + 2),
                            )
                        csl = bass.ts(ch, 512)
                        if half == 0:
                            if isq:
                                nc.vector.tensor_scalar(
                                    out=qTp[:, csl], in0=ps,
                                    scalar1=bqs_sb[:, hp:hp + 1],
                                    scalar2=float(SCALE),
                                    op0=mybir.AluOpType.add,
                                    op1=mybir.AluOpType.mult,
                                )
                            else:
                                nc.vector.tensor_scalar_add(
                                    out=kTp[:, csl], in0=ps,
                                    scalar1=bk_sb[:, hp:hp + 1],
                                )
                        else:
                            if isq:
                                nc.vector.scalar_tensor_tensor(
                                    out=qTp[:, csl], in0=ps,
                                    scalar=float(SCALE), in1=qTp[:, csl],
                                    op0=mybir.AluOpType.mult,
                                    op1=mybir.AluOpType.add,
                                )
                            else:
                                nc.vector.tensor_add(
                                    out=kTp[:, csl], in0=kTp[:, csl], in1=ps,
                                )
                    return f

                return {(isq, ch, half): mk(isq, ch, half)
                        for isq in (0, 1) for ch in range(4)
                        for half in (0, 1)}

            def emit_v_half(kt, half, xtb_sb, wv_sb):
                # v (normal layout, bf16) for heads 3*half..3*half+2 at kt;
                # wv moving operand is bf16 so the 192-wide output still
                # runs at 1 cycle/row
                psv = ps_pool.tile([128, DL // 2], f32, name="psv", tag="ps")
                vsl = bass.ds(half * (DL // 2), DL // 2)
                for dt in range(NDT):
                    nc.tensor.matmul(
                        psv,
                        xtb_sb[:, kt // 4, dt, bass.ds((kt % 4) * 128, 128)],
                        wv_sb[:, dt, vsl],
                        start=(dt == 0), stop=(dt == NDT - 1),
                    )
                nc.vector.tensor_copy(
                    out=v_sb[:, kt, 3 * half:3 * half + 3, :],
                    in_=psv.rearrange("p (h d) -> p h d", h=3),
                )

            use_act_copy = [False]

            def transpose_qt(hp, qtg):
                # oT[:, hp, qtg] = pack[:, qtg].T  (PE transpose via identity)
                oTps = ps_pool.tile([128, 128], bf16, name="oTps", tag="ps")
                nc.tensor.matmul(
                    oTps, pack_sb[:, qtg, :], eye_sb, is_transpose=True,
                )
                (nc.scalar.copy if use_act_copy[0] else
                 nc.vector.tensor_copy)(
                    out=oT_sb[:, hp, bass.ts(qtg, 128)], in_=oTps,
                )

            pend_S = {}

            def emit_S(h, qh, kt):
                # S^T matmuls for one kt step (run ahead of the exp stream
                # so a stalled AV can't head-of-line-block the next exp)
                hp, off = h // 2, (h % 2) * 64
                qTp, kTp = qk_tiles[hp]
                pss = pss_pool.tile([128, QHW], f32, name="pss", tag="pss")
                for c2 in range(QHW // 512):
                    nc.tensor.matmul(
                        pss[:, bass.ts(c2, 512)],
                        kTp[off:off + 64, bass.ts(kt, 128)],
                        qTp[off:off + 64, bass.ds(qh * QHW + c2 * 512, 512)],
                        start=True, stop=True,
                    )
                pend_S[(h, qh, kt)] = pss

            def emit_exp_av(h, qh, kt, mid_S=()):
                # exp of a pending S tile, then AV + denominator matmuls.
                # On the last kt all denominators go first so the reciprocal
                # chain starts early; mid_S splices later S emissions after
                # the first qt pair so a norm-blocked AV can't delay them.
                pss = pend_S.pop((h, qh, kt))
                pT = pT_pool.tile([128, QHW], bf16, name="pT", tag="pT")
                nc.scalar.activation(out=pT, in_=pss, func=ACT_EXP)

                def dn(qt):
                    nc.tensor.matmul(
                        pd_ps[:, qh, qt:qt + 1],
                        pT[:, bass.ts(qt, 128)],
                        ones_sb,
                        start=(kt == 0 and qt == 0),
                        stop=(kt == NKT - 1 and qt == QT - 1),
                    )

                def av(qt):
                    nc.tensor.matmul(
                        po_ps[:, qt, :],
                        pT[:, bass.ts(qt, 128)],
                        v_sb[:, kt, h, :],
                        start=(kt == 0 and qt == 0),
                        stop=(kt == NKT - 1 and qt == QT - 1),
                    )

                if kt == 0:
                    # mid-S splices, then the new qh's first AV/dn batch
                    # (the po/pd zeroing matmuls were emitted with the
                    # previous normalize)
                    for m in mid_S:
                        emit_S(*m)
                    for qt in range(QT):
                        dn(qt)
                        av(qt)
                elif kt == NKT - 1:
                    av(0)
                    for qt in range(QT):
                        dn(qt)
                    for m in mid_S:
                        emit_S(*m)
                    for qt in range(1, QT):
                        av(qt)
                else:
                    dn(0)
                    av(0)
                    for m in mid_S:
                        emit_S(*m)
                    for qt in range(1, QT):
                        dn(qt)
                        av(qt)



            def normalize_final(h, qh):
                # last normalize: ACT is idle, take half the copies there
                off2 = (h % 2) * 64
                rc = rcp_pool.tile([128, QT], f32, name="rc", tag="rc")
                nc.vector.reciprocal(out=rc, in_=pd_ps[:, qh, :])
                for qt in range(QT):
                    if qt % 2 == 0:
                        nc.vector.tensor_scalar_mul(
                            out=pack_sb[:, qh * QT + qt, off2:off2 + 64],
                            in0=po_ps[:, qt, :],
                            scalar1=rc[:, qt:qt + 1],
                        )
                    else:
                        nc.scalar.activation(
                            out=pack_sb[:, qh * QT + qt, off2:off2 + 64],
                            in_=po_ps[:, qt, :],
                            func=mybir.ActivationFunctionType.Copy,
                            scale=rc[:, qt:qt + 1],
                        )

            def normalize_qh(h, qh, zero_next=True):
                # per-partition softmax normalization into the pack tile,
                # split across DVE and Pool so the next qh's AVs unblock
                # fast.  The trailing full-range memsets order the next
                # qh's start=True matmuls (whose hardware zeroing covers
                # the whole 2KB bank region) after every normalize read,
                # via ordinary range-tracked WAR/WAW dependencies.
                off2 = (h % 2) * 64
                rc = rcp_pool.tile([128, QT], f32, name="rc", tag="rc")
                nc.vector.reciprocal(out=rc, in_=pd_ps[:, qh, :])
                for qt in range(QT):
                    nc.vector.tensor_scalar_mul(
                        out=pack_sb[:, qh * QT + qt, off2:off2 + 64],
                        in0=po_ps[:, qt, :],
                        scalar1=rc[:, qt:qt + 1],
                    )
                if zero_next:
                    nc.vector.memset(po_ps[:, :, :], 0.0)
                    nc.vector.memset(pd_ps[:, :, :], 0.0)

            def attend_head(h, fillers=None, next_first=None,
                            defer_qh1=False):
                # next_first: (h', qh', kt') whose S should be emitted during
                # this head's last step (cross-head lookahead).  defer_qh1
                # suppresses the cross-qh lookahead so qh1's first S is only
                # emitted at the boundary step (after qh1's qT chunks land).
                if (h, 0, 0) not in pend_S:
                    emit_S(h, 0, 0)

                def step_after(st, n):
                    hh, q2, k2 = st
                    for _ in range(n):
                        k2 += 1
                        if k2 == NKT:
                            k2 = 0
                            q2 += 1
                            if q2 == QH:
                                if next_first is None:
                                    return None
                                hh, q2, k2 = next_first
                    return (hh, q2, k2)

                for qh in range(QH):
                    for kt in range(NKT):
                        s = qh * NKT + kt
                        if fillers is not None and s < len(fillers):
                            for f in fillers[s]:
                                if f is not None:
                                    f()
                        if kt == 0:
                            # boundary step: AVs may wait on the previous
                            # qh's normalize; splice the next S emissions
                            # mid-step so the exp stream keeps running
                            if (h, qh, 0) not in pend_S:
                                emit_S(h, qh, 0)
                            mids = [step_after((h, qh, 0), n)
                                    for n in (1, 2, 3)]
                            mids = [m for m in mids
                                    if m is not None and m not in pend_S]
                            emit_exp_av(h, qh, kt, mid_S=mids)
                        else:
                            nxt = step_after((h, qh, kt), 1)
                            if defer_qh1 and nxt is not None and \
                                    nxt[0] == h and nxt[1] == 1 and \
                                    nxt[2] == 0 and qh == 0:
                                nxt = None
                            if nxt is not None and nxt not in pend_S:
                                emit_S(*nxt)
                            emit_exp_av(h, qh, kt)
                    normalize_qh(h, qh)

            ysh01 = {}

            def proj01_nh(qtg, nh, wp_sb):
                # dt<2 partial for one (q-tile, output-half); ship the tile
                # bf16 once both halves are staged
                if qtg not in ysh01:
                    ysh01[qtg] = ysh_pool.tile(
                        [128, EMBED], bf16, name="ysh", tag="ysh")
                ysh = ysh01[qtg]
                psy = ps_pool.tile([128, 384], f32, name="psy", tag="ps")
                for dt in range(2):
                    nc.tensor.matmul(
                        psy,
                        oT_sb[:, dt, bass.ts(qtg, 128)],
                        wp_sb[:, dt, bass.ts(nh, 384)],
                        start=(dt == 0), stop=(dt == 1),
                    )
                nc.vector.tensor_copy(out=ysh[:, bass.ts(nh, 384)], in_=psy)
                if nh == 1:
                    nc.sync.dma_start(
                        out=y_d.ap()[bass.ts(qtg, 128), :],
                        in_=ysh01.pop(qtg),
                    )

            def ship2(qtg, wp_sb, use_act=False):
                # dt=2 partial for one q-tile, shipped bf16 (host adds);
                # at the tail the ACT engine is idle and takes one copy,
                # and the dead pss pool's PSUM banks double the psy2 slots
                ysh = ysh_pool.tile([128, EMBED], bf16, name="ysh2", tag="ysh")
                for nh in range(2):
                    pool = pss_pool if (use_act and (qtg + nh) % 2 == 0) else ps_pool
                    psy2 = pool.tile([128, 384], f32, name="psy2",
                                     tag="pss" if pool is pss_pool else "ps")
                    nc.tensor.matmul(
                        psy2,
                        oT_sb[:, 2, bass.ts(qtg, 128)],
                        wp_sb[:, 2, bass.ts(nh, 384)],
                        start=True, stop=True,
                    )
                    if use_act and nh == 1:
                        nc.scalar.copy(out=ysh[:, bass.ts(nh, 384)], in_=psy2)
                    else:
                        nc.vector.tensor_copy(
                            out=ysh[:, bass.ts(nh, 384)], in_=psy2)
                nc.sync.dma_start(
                    out=y2_d.ap()[bass.ts(qtg, 128), :], in_=ysh,
                )

            # ---------------- phase A: qkv + heads 0..3 ----------------
            with tc.tile_pool(name="xw", bufs=1) as xw:
                xt_sb = xw.tile([128, 4, NDT, 512], f32r, name="xt_sb")
                xtb_sb = xw.tile([128, 4, NDT, 512], bf16, name="xtb_sb")
                wq_sb = xw.tile([128, NDT, DL], f32r, name="wq_sb")
                wk_sb = xw.tile([128, NDT, DL], f32r, name="wk_sb")
                wv_sb = xw.tile([128, NDT, DL], bf16, name="wv_sb")

                wq_r = wq_d.ap().rearrange("(n p) m -> p n m", p=128)
                wk_r = wk_d.ap().rearrange("(n p) m -> p n m", p=128)
                wv_r = wv_d.ap().rearrange("(n p) m -> p n m", p=128)

                # startup-critical weights on the SWDGE queue (their
                # transfers interleave with the xt stream), xt in
                # chunk-major pieces on the HWDGE queue: each DMA writes a
                # contiguous disjoint SBUF range so readers only depend on
                # the pieces they need
                nc.sync.dma_start(out=bqs_sb, in_=bqs_d.ap().rearrange("(n p) -> p n", p=128))
                nc.sync.dma_start(out=bk_sb, in_=bk_d.ap().rearrange("(n p) -> p n", p=128))
                nc.sync.dma_start(out=wq_sb[:, :, 0:128], in_=wq_r[:, :, 0:128])
                nc.sync.dma_start(out=wk_sb[:, :, 0:128], in_=wk_r[:, :, 0:128])
                nc.sync.dma_start(out=wv_sb, in_=wv_r)
                for ch in range(4):
                    for dh in range(2):
                        nc.sync.dma_start(
                            out=xt_sb[:, ch, 3 * dh:3 * dh + 3, :],
                            in_=xt_d.ap()[bass.ds(384 * dh, 384), bass.ts(ch, 512)]
                            .rearrange("(n p) m -> p n m", p=128),
                        )
                # bf16 shadow of xt for the V projection's stationary
                # operand (PE matmuls cannot mix 32-bit and 16-bit inputs)
                for ch in range(4):
                    for dh in range(2):
                        nc.gpsimd.tensor_copy(
                            out=xtb_sb[:, ch, 3 * dh:3 * dh + 3, :],
                            in_=xt_sb[:, ch, 3 * dh:3 * dh + 3, :],
                        )
                nc.sync.dma_start(out=wq_sb[:, :, 128:384], in_=wq_r[:, :, 128:384])
                nc.sync.dma_start(out=wk_sb[:, :, 128:384], in_=wk_r[:, :, 128:384])
                nc.sync.dma_start(out=auxf_sb, in_=aux_d.ap())
                nc.vector.tensor_copy(out=eye_sb, in_=auxf_sb[:, 0:128])
                nc.vector.tensor_copy(out=ones_sb, in_=auxf_sb[:, 128:129])

                e0 = qk_emitters(0, xt_sb, wq_sb, wk_sb)
                e1 = qk_emitters(1, xt_sb, wq_sb, wk_sb)
                e2 = qk_emitters(2, xt_sb, wq_sb, wk_sb)

                # upfront: pair-0 chunk 0, then V(0..3) + dep-free warm
                # matmuls to bridge the xt-chunk-1 DMA (keeps the PE busy so
                # queued matmuls aren't priced at a low p-state), then
                # chunk 1
                new_qk(0)
                e0[(1, 0, 0)]()
                e0[(1, 0, 1)]()
                e0[(0, 0, 0)]()
                e0[(0, 0, 1)]()
                for kt in range(4):
                    emit_v_half(kt, 0, xtb_sb, wv_sb)
                e0[(1, 1, 0)]()
                e0[(1, 1, 1)]()
                e0[(0, 1, 0)]()
                e0[(0, 1, 1)]()
                for kt in range(4, 8):
                    emit_v_half(kt, 0, xtb_sb, wv_sb)

                def sched(plan):
                    # plan: {step: [thunks]} -> filler list of NST entries
                    return [plan.get(s, []) for s in range(NST)]

                # head 0: V half-0 (kt 4-15; 0-3 done upfront); pair-0
                # k-chunks 2,3 early (kt deadlines), q-chunks 2,3 at the
                # qh boundary
                h0 = {s: [lambda kt=s: emit_v_half(kt, 0, xtb_sb, wv_sb)]
                      for s in range(8, NKT)}
                for s, em in [(1, (0, 2, 0)), (2, (0, 2, 1)),
                              (3, (0, 3, 0)), (4, (0, 3, 1)),
                              (10, (1, 2, 0)), (11, (1, 2, 1)),
                              (12, (1, 3, 0)), (13, (1, 3, 1))]:
                    h0.setdefault(s, []).append(e0[em])
                attend_head(0, sched(h0), next_first=(1, 0, 0),
                            defer_qh1=True)

                # head 1: pair-1 kT fully + qT chunks 0,1; V half-1 kt 0-7
                new_qk(1)
                h1 = {}
                for s, em in [(0, (0, 0, 0)), (2, (0, 0, 1)),
                              (4, (0, 1, 0)), (6, (0, 1, 1)),
                              (8, (1, 0, 0)), (10, (1, 0, 1)),
                              (11, (1, 1, 0)), (13, (1, 1, 1)),
                              (16, (0, 2, 0)), (18, (0, 2, 1)),
                              (20, (0, 3, 0)), (22, (0, 3, 1))]:
                    h1.setdefault(s, []).append(e1[em])
                for i, s in enumerate((17, 19, 21, 23, 25, 27, 29, 31)):
                    h1.setdefault(s, []).append(
                        lambda kt=i: emit_v_half(kt, 1, xtb_sb, wv_sb))
                attend_head(1, sched(h1), next_first=(2, 0, 0))

                # head 2: pair-1 qT chunks 2,3; pair-2 kT + qT chunks 0,1;
                # pair-0 transposes on otherwise-light steps
                new_qk(2)
                h2 = {}
                for s, em in [(2, (1, 2, 0)), (4, (1, 2, 1)),
                              (6, (1, 3, 0)), (8, (1, 3, 1))]:
                    h2.setdefault(s, []).append(e1[em])
                for s, em in [(10, (0, 0, 0)), (12, (0, 0, 1)),
                              (14, (0, 1, 0)), (16, (0, 1, 1)),
                              (18, (1, 0, 0)), (20, (1, 0, 1)),
                              (22, (1, 1, 0)), (24, (1, 1, 1)),
                              (26, (0, 2, 0)), (28, (0, 2, 1)),
                              (29, (0, 3, 0)), (30, (0, 3, 1))]:
                    h2.setdefault(s, []).append(e2[em])
                for i, s in enumerate((17, 19, 21, 23)):
                    h2.setdefault(s, []).append(
                        lambda kt=8 + i: emit_v_half(kt, 1, xtb_sb, wv_sb))
                for qtg in range(NQT):
                    s = 2 * qtg + (0 if qtg < 8 else 1)
                    h2.setdefault(s, []).append(
                        lambda q=qtg: transpose_qt(0, q))
                attend_head(2, sched(h2), next_first=(3, 0, 0))

                # head 3: pair-2 qT chunks 2,3; V half-1 kt 8-15
                h3 = {}
                for s, em in [(2, (1, 2, 0)), (4, (1, 2, 1)),
                              (6, (1, 3, 0)), (8, (1, 3, 1))]:
                    h3.setdefault(s, []).append(e2[em])
                for i, s in enumerate((1, 3, 5, 7)):
                    h3.setdefault(s, []).append(
                        lambda kt=12 + i: emit_v_half(kt, 1, xtb_sb, wv_sb))
                attend_head(3, sched(h3), next_first=(4, 0, 0))

            # ---------------- phase B: heads 4,5 + projection ----------
            with tc.tile_pool(name="yb", bufs=1) as yb:
                wp_sb = yb.tile([128, 3, EMBED], bf16, name="wp_sb")
                nc.gpsimd.dma_start(
                    out=wp_sb, in_=wp_d.ap().rearrange("(n p) m -> p n m", p=128))

                # head 4: pair-1 transposes + proj01 ships (from step 4,
                # after the wp DMA has landed)
                p01 = [(q, nh) for q in range(NQT) for nh in range(2)]
                h4 = {}
                for qtg in range(NQT):
                    h4.setdefault(2 * qtg + 1 - (qtg // 8), []).append(
                        lambda q=qtg: transpose_qt(1, q))
                for i, s in enumerate(range(2, NST)):
                    q, nh = p01[i]
                    h4.setdefault(s, []).append(
                        lambda q=q, nh=nh: proj01_nh(q, nh, wp_sb))
                attend_head(4, [h4.get(s, []) for s in range(NST)],
                            next_first=(5, 0, 0))

                # head 5 qh0: finish proj01 remainder (4 ships)
                rem = p01[NST - 2:]
                for kt in range(NKT):
                    if kt < len(rem):
                        q, nh = rem[kt]
                        proj01_nh(q, nh, wp_sb)
                    if kt == 0:
                        emit_exp_av(5, 0, 0, mid_S=[(5, 0, 1), (5, 0, 2)])
                    else:
                        nxt = (5, 0, kt + 1) if kt + 1 < NKT else (5, 1, 0)
                        if nxt not in pend_S:
                            emit_S(*nxt)
                        emit_exp_av(5, 0, kt)
                normalize_qh(5, 0)

                # head 5 qh1: pair-2 transposes qtg 0..7 (odd steps) and
                # dt=2 ships qtg 0..6 (even steps from 2)
                for kt in range(NKT):
                    if kt % 2 == 0:
                        transpose_qt(2, kt // 2)
                    if kt % 2 == 1 and kt >= 3:
                        ship2(kt // 2 - 1, wp_sb)
                    if kt == NKT - 1:
                        ship2(QT - 1, wp_sb)
                    if kt == 0:
                        emit_exp_av(5, 1, 0, mid_S=[(5, 1, 1), (5, 1, 2)])
                    else:
                        if kt + 1 < NKT and (5, 1, kt + 1) not in pend_S:
                            emit_S(5, 1, kt + 1)
                        emit_exp_av(5, 1, kt)
                normalize_final(5, 1)

                # tail: ship the already-transposed qtg 7 first, then
                # interleave transpose/ship per tile (deep pools let the
                # engines pipeline across tiles)
                for qtg in range(QT, NQT):
                    use_act_copy[0] = (qtg % 2 == 0)
                    transpose_qt(2, qtg)
                    use_act_copy[0] = False
                    if qtg > QT:
                        ship2(qtg - 1, wp_sb, use_act=True)
                ship2(NQT - 1, wp_sb, use_act=True)

    nc.finalize()
    return nc


def _shard_inputs(x, w_qkv, b_qkv, w_proj):
    import ml_dtypes

    aux = np.zeros((128, 129), dtype=np.float32)
    aux[:, 0:128] = np.eye(128, dtype=np.float32)
    aux[:, 128] = 1.0
    in_maps = []
    for c in range(NCORES):
        b, g = c // 2, c % 2
        sl = slice(DL * g, DL * g + DL)
        in_maps.append({
            "xt": np.ascontiguousarray(x[b].T),
            "wq": np.ascontiguousarray(w_qkv[:, sl]),
            "wk": np.ascontiguousarray(w_qkv[:, EMBED:][:, sl]),
            "wv": np.ascontiguousarray(w_qkv[:, 2 * EMBED:][:, sl]).astype(ml_dtypes.bfloat16),
            "bqs": np.ascontiguousarray(b_qkv[sl]),
            "bk": np.ascontiguousarray(b_qkv[EMBED:][sl]),
            "wp": np.ascontiguousarray(w_proj[sl, :]).astype(ml_dtypes.bfloat16),
            "aux": aux,
        })
    return in_maps


def kernel(x, w_qkv, b_qkv, w_proj, b_proj, _profile=False, _repeat=1):
    from concourse.bass_utils import run_bass_kernel_spmd

    x = np.asarray(x, dtype=np.float32)
    w_qkv = np.asarray(w_qkv, dtype=np.float32)
    b_qkv = np.asarray(b_qkv, dtype=np.float32)
    w_proj = np.asarray(w_proj, dtype=np.float32)
    b_proj = np.asarray(b_proj, dtype=np.float32)

    if _repeat not in _prog_cache:
        _prog_cache[_repeat] = _build_program(_repeat)
    nc = _prog_cache[_repeat]

    in_maps = _shard_inputs(x, w_qkv, b_qkv, w_proj)
    res = run_bass_kernel_spmd(
        nc, in_maps, list(range(NCORES)), trace=_profile,
    )

    # host-side gather: sum the dt<2 / dt=2 partials of the two head-group
    # cores per batch and add the bias row (v-bias folded through w_proj,
    # plus b_proj itself)
    bias_row = b_qkv[2 * EMBED:] @ w_proj + b_proj
    y = np.empty((B, T, EMBED), dtype=np.float32)
    for b in range(B):
        acc = np.broadcast_to(bias_row.astype(np.float32), (T, EMBED)).copy()
        for c in (2 * b, 2 * b + 1):
            acc += np.asarray(res.results[c]["y"], dtype=np.float32)
            acc += np.asarray(res.results[c]["y2"], dtype=np.float32)
        y[b] = acc
    if _profile:
        return y, res
    return y
